# revision 4
# baseline (speedup 1.0000x reference)
"""Trainium2 Bass kernel for per-edge-type Linear + ReLU (GNN message passing).

out[e] = relu(edge_features[e] @ W[edge_types[e]] + b[edge_types[e]])
E = 1M edges, D_in = D_out = 64, 8 edge types, 8 NeuronCores.

Strategy (data-parallel over edges, weights replicated):
  - Shard edges 8 ways on the host; each core gets E/8 edges.
  - Host-side layout prep only (transpose / cast / pad / replicate):
      * X^T in fp16 [64, Ec] so the matmul stationary (lhsT, K=features on
        partitions) DMA-loads directly.
      * edge types replicated to 8 partitions in fp16 [8, Ec] so the device
        can build the one-hot O^T with a single is_equal against an iota
        constant.
  - Per 128-edge tile, on device:
      * MM1: lhsT = X^T tile [64,128] fp16, rhs = W_all [64, 512] fp16
        (all 8 type weights side by side) -> PSUM Z [128 edges, 8*64].
      * MM2 (accumulate): lhsT = O^T tile [8,128] fp16, rhs = Bfull [8,512]
        where Bfull[s, t*64+j] = b[t,j] if s==t else -30000.  This adds the
        per-edge bias to the correct 64-col block and pushes every
        wrong-type block to ~-30000.
      * One DVE tensor_reduce(max) over the type axis collapses [128, 8, 64]
        -> [128, 64] = selected result (+bias).  A trailing in-place
        max(x, 0) is the ReLU (wrong blocks are hugely negative, so they
        never win; if the selected value is negative, relu clamps it).
  - Store [128, 64] fp32 tiles back to the natural [E, 64] layout.
"""

import os
from contextlib import ExitStack

import numpy as np

import concourse.bacc as bacc
import concourse.bass as bass
import concourse.mybir as mybir
import concourse.tile as tile
from concourse.bass_utils import run_bass_kernel_spmd

E_TOTAL = 1_000_000
D = 64
N_TYPES = 8
N_CORES = 8
TILE_E = 128            # edges per matmul tile (PSUM partition dim)
G = 8                   # tiles per DMA macro-block
BLOCK_E = TILE_E * G    # 1024 edges per block
NEG_PENALTY = -30000.0  # wrong-type block offset (fp16-exact, dwarfs |X@W|)

_BUILD_CACHE: dict = {}
LAST_RESULTS = None     # BassKernelResults from the most recent run (for test.py)


def _build_program(ec_pad: int):
    """Build + compile the single-core Bass program (same on all 8 cores)."""
    nblk = ec_pad // BLOCK_E
    f16 = mybir.dt.float16
    f32 = mybir.dt.float32

    nc = bacc.Bacc("TRN2", target_bir_lowering=False, debug=False)

    xt = nc.dram_tensor("xt", [D, ec_pad], f16, kind="ExternalInput").ap()
    ty = nc.dram_tensor("ty", [N_TYPES, ec_pad], f16, kind="ExternalInput").ap()
    w_all = nc.dram_tensor("w_all", [D, N_TYPES * D], f16, kind="ExternalInput").ap()
    bfull = nc.dram_tensor("bfull", [N_TYPES, N_TYPES * D], f16, kind="ExternalInput").ap()
    iota_t = nc.dram_tensor("iota_t", [N_TYPES, BLOCK_E], f16, kind="ExternalInput").ap()
    out = nc.dram_tensor("out", [ec_pad, D], f32, kind="ExternalOutput").ap()

    with tile.TileContext(nc) as tc, ExitStack() as ctx:
        const_pool = ctx.enter_context(tc.tile_pool(name="consts", bufs=1))
        xt_pool = ctx.enter_context(tc.tile_pool(name="xt", bufs=3))
        ty_pool = ctx.enter_context(tc.tile_pool(name="ty", bufs=3))
        ot_pool = ctx.enter_context(tc.tile_pool(name="ot", bufs=3))
        out_pool = ctx.enter_context(tc.tile_pool(name="outs", bufs=3))
        z_pool = ctx.enter_context(tc.tile_pool(name="z", bufs=8, space="PSUM"))

        w_sb = const_pool.tile([D, N_TYPES * D], f16)
        nc.sync.dma_start(w_sb[:], w_all)
        bf_sb = const_pool.tile([N_TYPES, N_TYPES * D], f16)
        nc.sync.dma_start(bf_sb[:], bfull)
        io_sb = const_pool.tile([N_TYPES, BLOCK_E], f16)
        nc.sync.dma_start(io_sb[:], iota_t)

        for blk in range(nblk):
            sl = slice(blk * BLOCK_E, (blk + 1) * BLOCK_E)
            xt_t = xt_pool.tile([D, BLOCK_E], f16, tag="xt")
            nc.sync.dma_start(xt_t[:], xt[:, sl])
            ty_t = ty_pool.tile([N_TYPES, BLOCK_E], f16, tag="ty")
            nc.sync.dma_start(ty_t[:], ty[:, sl])

            # One-hot over types, transposed: ot[s, e] = (type[e] == s)
            ot_t = ot_pool.tile([N_TYPES, BLOCK_E], f16, tag="ot")
            nc.vector.tensor_tensor(
                ot_t[:], ty_t[:], io_sb[:], mybir.AluOpType.is_equal
            )

            out_t = out_pool.tile([TILE_E, G, D], f32, tag="outs")
            for j in range(G):
                js = slice(j * TILE_E, (j + 1) * TILE_E)
                z = z_pool.tile([TILE_E, N_TYPES * D], f32, tag="z")
                nc.tensor.matmul(
                    z[:], lhsT=xt_t[:, js], rhs=w_sb[:], start=True, stop=False
                )
                nc.tensor.matmul(
                    z[:], lhsT=ot_t[:, js], rhs=bf_sb[:], start=False, stop=True
                )
                # Collapse the 8 candidate blocks: max over the type axis.
                z_red = z[:].rearrange("p (t j) -> p j t", t=N_TYPES)
                nc.vector.tensor_reduce(
                    out_t[:, j, :], z_red, mybir.AxisListType.X, mybir.AluOpType.max
                )
            # ReLU in place over the whole block.
            nc.vector.tensor_scalar_max(out_t[:], out_t[:], 0.0)

            out_view = out[sl, :].rearrange("(g p) d -> p g d", p=TILE_E)
            nc.sync.dma_start(out_view, out_t[:])

    nc.compile()
    return nc


def _get_program(ec_pad: int):
    if ec_pad not in _BUILD_CACHE:
        _BUILD_CACHE[ec_pad] = _build_program(ec_pad)
    return _BUILD_CACHE[ec_pad]


def build_in_maps(edge_features, edge_types, W, b):
    e_total = edge_features.shape[0]
    assert e_total % N_CORES == 0
    ec = e_total // N_CORES
    nblk = (ec + BLOCK_E - 1) // BLOCK_E
    ec_pad = nblk * BLOCK_E

    # Shared (replicated) operands.
    w_all = np.ascontiguousarray(
        np.asarray(W, dtype=np.float32).transpose(1, 0, 2).reshape(D, N_TYPES * D)
    ).astype(np.float16)
    bfull = np.full((N_TYPES, N_TYPES * D), NEG_PENALTY, dtype=np.float16)
    for t in range(N_TYPES):
        bfull[t, t * D : (t + 1) * D] = np.asarray(b[t], dtype=np.float32)
    iota_t = np.broadcast_to(
        np.arange(N_TYPES, dtype=np.float16)[:, None], (N_TYPES, BLOCK_E)
    ).copy()

    x = np.asarray(edge_features, dtype=np.float32)
    t_f16 = np.asarray(edge_types).astype(np.float16)

    in_maps = []
    for c in range(N_CORES):
        sl = slice(c * ec, (c + 1) * ec)
        xt = np.zeros((D, ec_pad), dtype=np.float16)
        xt[:, :ec] = x[sl].T.astype(np.float16)
        ty = np.zeros((N_TYPES, ec_pad), dtype=np.float16)
        ty[:, :ec] = t_f16[sl][None, :]
        in_maps.append(
            {"xt": xt, "ty": ty, "w_all": w_all, "bfull": bfull, "iota_t": iota_t}
        )
    return in_maps


def kernel(edge_features, edge_types, W, b):
    global LAST_RESULTS
    e_total = edge_features.shape[0]
    ec = e_total // N_CORES
    nblk = (ec + BLOCK_E - 1) // BLOCK_E
    ec_pad = nblk * BLOCK_E

    nc = _get_program(ec_pad)
    in_maps = build_in_maps(edge_features, edge_types, W, b)

    res = run_bass_kernel_spmd(
        nc,
        in_maps,
        core_ids=list(range(N_CORES)),
        trace=bool(int(os.environ.get("EDGE_KERNEL_TRACE", "0"))),
    )
    LAST_RESULTS = res

    out = np.empty((e_total, D), dtype=np.float32)
    for c in range(N_CORES):
        out[c * ec : (c + 1) * ec] = res.results[c]["out"][:ec]
    return out
